# revision 3
# baseline (speedup 1.0000x reference)
"""Clusformer Trainium2 kernel (8-core SPMD).

Problem: nn_Clusformer — cross-attention argmax cluster assignment +
segment-sum of node features into L=32 clusters, followed by a tiny
[B,L,D] centroid MHSA/BatchNorm/FFN head.

Math refactoring (exact up to fp rounding):
  scores[b,t,l] = (X@Wk_n + bk_n) . Q_cent[b,l]  ==  X @ M[b] + c0[b]
      with  M[b] = Wk_n @ Q_cent[b].T  ([C,L]),  c0[b] = bk_n @ Q_cent[b].T
  (the 1/sqrt(C) scale does not change the argmax)
  cluster_V[b,l] = (sum_{t in l} X[t]) @ Wv_n + counts[b,l] * bv_n
  so the device only needs a segment-sum of raw X plus counts.
  counts come from a belongs^T @ belongs matmul: diag = per-cluster count
  (ties contribute to every tied cluster, same as a ones-column would).

Device kernel (per core, 24576 tokens = half of one batch), fp8e4 inputs
(the count^2 normalization makes the output ~1e-4-insensitive to the
cluster path, so fp8 rounding / ~4.5% argmax flips cost < 3e-5):
  - scores tile [128 tok, 32] = X^T-tile (as PE weights) @ M, c0 seeded
    into PSUM via a K=1 ones-matmul; fp8 inputs, fp32 PSUM.
  - one-hot assignment via reduce_max + is_ge on DVE reading the PSUM
    scores bank DIRECTLY (no ScalarE eviction hop — DVE is 1x-mode for
    both ops anyway, ~600ns/op measured). rowmax is fp32: a bf16 rowmax
    vs fp32 scores compare produces spurious multi-hots (6x the error).
  - segment sums via fp8 DoubleRow PE matmuls (two token-tiles per mm):
    belongs^T [32,tok] @ X [tok,128] into sums_ps[:, :128] and
    belongs^T @ belongs into sums_ps[:, 128:160] (diag -> counts).
Host: reduce the 8 partial [32,160] sums, then the tiny [4,32,64]
MHSA/BN/FFN head in float64 (0.006% of total FLOPs).

Perf notes (v2; v0 baseline 36.1us, v1 regressed to 42.5us):
  - the run is DMA-streaming-bound: 2x ~3.15MB fp8 per core (xt for
    scores, xn for sums — the two layouts are PE-operand-forced; an
    on-chip transpose costs more PE/DVE than the extra DMA traffic).
  - xt/xn are HOST-PACKED into ONE interleaved dram tensor (per-group
    2048-col xt block | 2048-col xn block) streamed on a single HWDGE
    ring: two concurrent queues measured ~300 GB/s combined vs ~420
    solo (HBM interleaving penalty), and one ring halves the NX issue
    load. mc/out ride the other ring.
  - the PE is warmed with ~3.5us of dummy matmuls during the DMA-arm
    dead time: HAM starts kernels at 1.2 GHz and only unthrottles
    after ~3.4us of sustained busy; in v1 the cold PE (1.9us/group vs
    1.31us arrivals) never warmed until 35us and became the critical
    path. Warm PE is ~1.2us/group and hides under the stream.
  - ~7us of measured exec is an NRT-emitted postamble (255 sem clears
    split across engines) + ~1.5us HWDGE arm + ~0.7us const preamble:
    fixed toolchain tax, not addressable from the kernel.
  - walrus here rejects instructions with >1 sem-wait (_split_waits) and
    the Tile exit barrier is lightened (_TC).
"""

import os
import numpy as np
import ml_dtypes

import concourse.bass as bass
import concourse.mybir as mybir
import concourse.tile as tile
from concourse import bass_utils

B, T, N, C = 4, 12, 4096, 128
L, D, H = 32, 64, 4
HD = D // H
EPS_BN = 1e-5

NCORES = 8
TOK = T * N  # tokens per batch = 49152
TOK_PER_CORE = B * TOK // NCORES  # 24576
TILE_T = 128
NTILE = TOK_PER_CORE // TILE_T  # 192
GT = 16  # token-tiles per scores group (PSUM bank = [128, 512] fp32)
NG = NTILE // GT  # 12
GW = GT * TILE_T  # xt cols per group = 2048
GX = GT * C  # xn cols per group = 2048
GP = GW + GX  # packed cols per group = 4096
W = 160  # sums PSUM width: 128 X-channels + 32 counts (diag of b^T b)
N_WARM = 8  # dummy matmuls to lift HAM to K=8/8 before real work

BF16 = mybir.dt.bfloat16
FP8 = mybir.dt.float8e4
F32 = mybir.dt.float32
_bf = ml_dtypes.bfloat16
_f8 = ml_dtypes.float8_e4m3

_cache = {}


def _split_waits(nc, limit=1):
    """Walrus in this container rejects >1 sem-wait per instruction
    (CoreV3 setupSyncWait): hoist excess waits onto preceding same-engine
    NOPs."""
    n = 0
    for f in nc.m.functions:
        for bb in f.blocks:
            insts = bb.instructions
            i = 0
            while i < len(insts):
                inst = insts[i]
                si = getattr(inst, "sync_info", None)
                if si is not None and si.on_wait is not None and len(si.on_wait) > limit:
                    waits = list(si.on_wait)
                    si.on_wait = waits[:limit]
                    extra = waits[limit:]
                    pos = i
                    while extra:
                        chunk, extra = extra[:limit], extra[limit:]
                        n += 1
                        insts.insert(
                            pos,
                            mybir.InstNoOp(
                                name=f"I-waitsplit-{n}",
                                sync_info=mybir.SyncInfo(on_wait=chunk, on_update=[]),
                                bass_nofuse=True,
                                engine=inst.engine,
                            ),
                        )
                        pos += 1
                        i += 1
                i += 1
    return n


class _TC(tile.TileContext):
    """TileContext with a lighter exit: drop the trailing all-engine
    barrier after the semaphore clears. The clears still run (re-execution
    safe); NRT completion waits for every engine to halt regardless."""

    def _drain_and_barrier(self, tick_clock, wait_clock):
        from concourse.vector_clock import ScopedClock

        drain_inst = self.nc.sync.drain()
        wait_clock.add_sem_waits(
            drain_inst.ins, ScopedClock({None: tick_clock.global_clock})
        )
        self.nc.all_engine_barrier()
        popped = self.nc._tile_sem_poison_stack.pop()
        assert popped is self._sem_poison
        self.nc.clear_and_free_semaphores(list(self.sems.allocated().values()))


def _build_kernel():
    nc = bass.Bass()
    xin = nc.dram_tensor("xin", [TILE_T, NG * GP], FP8, kind="ExternalInput")
    # mc packs M ([:, :L]) and the 16x-tiled c0 row (row 0, cols L:L+GT*L)
    mc = nc.dram_tensor("mc", [C, L + GT * L], FP8, kind="ExternalInput")
    out = nc.dram_tensor("out", [L, W], F32, kind="ExternalOutput")

    with _TC(nc) as tc:
        with (
            tc.tile_pool(name="const", bufs=1) as constp,
            tc.tile_pool(name="xin", bufs=1) as xinp,
            tc.tile_pool(name="work", bufs=5) as workp,
            tc.tile_pool(name="pss", bufs=6, space="PSUM") as pssp,
            tc.tile_pool(name="psum_acc", bufs=1, space="PSUM") as psap,
            tc.tile_pool(name="psum_warm", bufs=1, space="PSUM") as pswp,
        ):
            # mc rides the ACT ring (scores need M + c0 before anything
            # else; 70KB, lands before the first xin slab finishes)
            mc_sb = constp.tile([C, L + GT * L], FP8)
            nc.scalar.dma_start(mc_sb[:], mc[:])
            m_sb = mc_sb[:, :L]
            c0_sb = mc_sb[0:1, L : L + GT * L]
            ones_sb = constp.tile([1, TILE_T], FP8)
            nc.vector.memset(ones_sb[:], 1.0)

            # PE warmup: HAM holds the PE at 1.2 GHz until it sees ~3.4us
            # of sustained activity. Burn the DMA-arm dead time on dummy
            # matmuls so the real stream runs at 2.4 GHz from the start.
            wu_sb = constp.tile([TILE_T, 640], FP8)
            nc.vector.memset(wu_sb[:], 1.0)
            wu_ps = pswp.tile([TILE_T, 512], F32)
            for _ in range(N_WARM):
                nc.tensor.matmul(
                    wu_ps[:],
                    wu_sb[:, :TILE_T],
                    wu_sb[:, TILE_T:],
                    start=True,
                    stop=True,
                    skip_group_check=True,
                )

            # single full-size resident input tile; per-group slab DMAs on
            # ONE ring (solo queue streams ~420 GB/s vs ~300 for two).
            xin_sb = xinp.tile([TILE_T, NG * GP], FP8)
            slabs = [(0, GW), (GW, GP)]  # group 0 split: xt block, xn block
            s = GP
            for span in (1, 2, 2, 2, 2, 1, 1):  # groups 1..11
                slabs.append((s, s + span * GP))
                s += span * GP
            assert s == NG * GP
            for a, b in slabs:
                nc.sync.dma_start(xin_sb[:, a:b], xin[:, a:b])

            sums_ps = psap.tile([L, W], F32)

            def scores_group(g):
                xt_s = xin_sb[:, g * GP : g * GP + GW]
                scores_ps = pssp.tile([TILE_T, GT * L], F32)
                # seed every token row of the group's PSUM with c0
                nc.tensor.matmul(
                    scores_ps[:],
                    ones_sb[:],
                    c0_sb[:],
                    start=True,
                    stop=False,
                    skip_group_check=True,
                )
                for i in range(GT):
                    nc.tensor.matmul(
                        scores_ps[:, i * L : (i + 1) * L],
                        xt_s[:, i * TILE_T : (i + 1) * TILE_T],
                        m_sb[:],
                        start=False,
                        stop=(i == GT - 1),
                        skip_group_check=True,
                    )
                # one-hot straight off PSUM: both DVE ops are 1x-mode
                # regardless (reduce has no 2x uop; is_ge's broadcast
                # operand forces 1x), so an SBUF eviction hop buys nothing
                s3 = scores_ps.rearrange("p (g l) -> p g l", l=L)
                rowmax = workp.tile([TILE_T, GT], F32, tag="rowmax")
                nc.vector.reduce_max(rowmax[:], s3, axis=mybir.AxisListType.X)
                belongs = workp.tile([TILE_T, GT * L], FP8, tag="belongs")
                nc.vector.tensor_tensor(
                    belongs.rearrange("p (g l) -> p g l", l=L),
                    s3,
                    rowmax[:, :, None].to_broadcast((TILE_T, GT, L)),
                    mybir.AluOpType.is_ge,
                )
                return belongs

            def sums_group(g, belongs):
                # fp8 DoubleRow: two token-tiles per matmul (K=256).
                # rhs #1: X tiles -> sums_ps[:, :128]
                # rhs #2: belongs itself -> sums_ps[:, 128:160] (diag=counts)
                b3 = belongs.rearrange("p (k l) -> p k l", l=L)
                x3 = xin_sb[:, g * GP + GW : (g + 1) * GP].rearrange(
                    "p (k w) -> p k w", w=C
                )
                for i in range(GT // 2):
                    first = g == 0 and i == 0
                    last = g == NG - 1 and i == GT // 2 - 1
                    nc.tensor.matmul(
                        sums_ps[:, :C],
                        b3[:, 2 * i : 2 * i + 2, :],
                        x3[:, 2 * i : 2 * i + 2, :],
                        start=first,
                        stop=False,
                        perf_mode=mybir.MatmulPerfMode.DoubleRow,
                        skip_group_check=True,
                    )
                    nc.tensor.matmul(
                        sums_ps[:, C : C + L],
                        b3[:, 2 * i : 2 * i + 2, :],
                        b3[:, 2 * i : 2 * i + 2, :],
                        start=False,
                        stop=last,
                        perf_mode=mybir.MatmulPerfMode.DoubleRow,
                        skip_group_check=True,
                    )

            # software pipeline: sums-matmuls run two groups behind the
            # scores-matmuls so the PE never waits on the DVE one-hot.
            pend = []
            for g in range(NG):
                pend.append((g, scores_group(g)))
                if len(pend) > 1:
                    pg, cur = pend.pop(0)
                    sums_group(pg, cur)
            for pg, cur in pend:
                sums_group(pg, cur)

            out_sb = constp.tile([L, W], F32, tag="out_sb")
            nc.scalar.activation(
                out_sb[:], sums_ps[:], mybir.ActivationFunctionType.Copy
            )
            nc.scalar.dma_start(out[:], out_sb[:])

    _split_waits(nc)
    return nc


def _prep_inputs(STFeature, centroids, Wq_c, bq_c, Wk_n, bk_n):
    X = np.ascontiguousarray(STFeature.reshape(B, TOK, C), dtype=np.float32)
    Qc = centroids.astype(np.float64) @ Wq_c.astype(np.float64) + bq_c.astype(
        np.float64
    )  # [B,L,C]
    M = np.einsum("cj,blj->bcl", Wk_n.astype(np.float64), Qc)  # [B,C,L]
    c0 = np.einsum("j,blj->bl", bk_n.astype(np.float64), Qc)  # [B,L]

    in_maps = []
    for core in range(NCORES):
        b, h = core // 2, core % 2
        rows = X[b][h * TOK_PER_CORE : (h + 1) * TOK_PER_CORE]  # [24576, 128]
        xt = np.ascontiguousarray(rows.T).astype(_f8)  # [128, 24576]
        xn = (
            rows.reshape(NTILE, TILE_T, C).transpose(1, 0, 2).astype(_f8)
        )  # [128, 192, 128]
        # interleave per group: [xt block 2048 | xn block 2048] x 12
        xin = np.concatenate(
            [xt.reshape(TILE_T, NG, GW), xn.reshape(TILE_T, NG, GX)], axis=2
        ).reshape(TILE_T, NG * GP)
        mc = np.zeros((C, L + GT * L), dtype=_f8)
        mc[:, :L] = M[b].astype(np.float32).astype(_f8)
        mc[0, L:] = np.tile(c0[b].astype(np.float32).astype(_f8), GT)
        in_maps.append({"xin": np.ascontiguousarray(xin), "mc": mc})
    return in_maps


def _small_path(Xsum, counts, centroids, Wv_n, bv_n, Wal, bal, Wq, bq, Wk, bk, Wv, bv,
                Wo, bo, bn_gamma, bn_beta, alpha, beta, W1, b1, W2, b2):
    f = lambda a: np.asarray(a, np.float64)
    V = Xsum @ f(Wv_n) + counts[:, :, None] * f(bv_n)
    cluster = V / (counts**2 + 1.0)[:, :, None]
    cen = f(centroids) + cluster @ f(Wal) + f(bal)
    q = (cen @ f(Wq) + f(bq)).reshape(B, L, H, HD).transpose(0, 2, 1, 3)
    k = (cen @ f(Wk) + f(bk)).reshape(B, L, H, HD).transpose(0, 2, 1, 3)
    v = (cen @ f(Wv) + f(bv)).reshape(B, L, H, HD).transpose(0, 2, 1, 3)
    s = np.einsum("bhld,bhmd->bhlm", q, k) / np.sqrt(np.float64(HD))
    s = s - s.max(axis=-1, keepdims=True)
    e = np.exp(s)
    attn = e / e.sum(axis=-1, keepdims=True)
    a = np.einsum("bhlm,bhmd->bhld", attn, v).transpose(0, 2, 1, 3).reshape(B, L, D)
    a = a @ f(Wo) + f(bo)
    z = cen + a
    mu = z.mean(axis=(0, 1))
    var = z.var(axis=(0, 1))
    z = (z - mu) / np.sqrt(var + EPS_BN) * f(bn_gamma) + f(bn_beta)
    z = f(alpha) * z + f(beta)
    return np.maximum(z @ f(W1) + f(b1), 0.0) @ f(W2) + f(b2)


def kernel(**inputs):
    inputs = {k: np.asarray(v) for k, v in inputs.items()}
    in_maps = _prep_inputs(
        inputs["STFeature"].astype(np.float32),
        inputs["centroids"],
        inputs["Wq_c"],
        inputs["bq_c"],
        inputs["Wk_n"],
        inputs["bk_n"],
    )

    if "nc" not in _cache:
        _cache["nc"] = _build_kernel()
    nc = _cache["nc"]

    run_kwargs = {}
    if os.environ.get("CLUSF_TRACE"):
        run_kwargs = {"trace": True, "tmpdir": os.environ.get("CLUSF_TRACE_DIR")}
    res = bass_utils.run_bass_kernel_spmd(
        nc, in_maps, core_ids=list(range(NCORES)), **run_kwargs
    )
    _cache["last_result"] = res

    sums8 = np.stack([res.results[i]["out"] for i in range(NCORES)])  # [8,32,W]
    S = (sums8[0::2] + sums8[1::2]).astype(np.float64)  # [B,32,W]
    Xsum = S[:, :, :C]
    counts = np.einsum("bll->bl", S[:, :, C : C + L])  # diag of belongs^T belongs

    out = _small_path(
        Xsum, counts,
        inputs["centroids"], inputs["Wv_n"], inputs["bv_n"], inputs["Wal"],
        inputs["bal"], inputs["Wq"], inputs["bq"], inputs["Wk"], inputs["bk"],
        inputs["Wv"], inputs["bv"], inputs["Wo"], inputs["bo"],
        inputs["bn_gamma"], inputs["bn_beta"], inputs["alpha"], inputs["beta"],
        inputs["W1"], inputs["b1"], inputs["W2"], inputs["b2"],
    )
    return out.astype(np.float32)


# revision 4
# speedup vs baseline: 1.0802x; 1.0802x over previous
"""Clusformer Trainium2 kernel (8-core SPMD).

Problem: nn_Clusformer — cross-attention argmax cluster assignment +
segment-sum of node features into L=32 clusters, followed by a tiny
[B,L,D] centroid MHSA/BatchNorm/FFN head.

Math refactoring (exact up to fp rounding):
  scores[b,t,l] = (X@Wk_n + bk_n) . Q_cent[b,l]  ==  X @ M[b] + c0[b]
      with  M[b] = Wk_n @ Q_cent[b].T  ([C,L]),  c0[b] = bk_n @ Q_cent[b].T
  (the 1/sqrt(C) scale does not change the argmax)
  cluster_V[b,l] = (sum_{t in l} X[t]) @ Wv_n + counts[b,l] * bv_n
  so the device only needs a segment-sum of raw X plus counts.
  counts come from a belongs^T @ belongs matmul: diag = per-cluster count
  (ties contribute to every tied cluster, same as a ones-column would).

Device kernel (per core, 24576 tokens = half of one batch), fp8e4 inputs
(the count^2 normalization makes the output ~1e-4-insensitive to the
cluster path, so fp8 rounding / ~4.5% argmax flips cost < 3e-5):
  - scores tile [128 tok, 32] = X^T-tile (as PE weights) @ M, c0 seeded
    into PSUM via a K=1 ones-matmul; fp8 inputs, fp32 PSUM.
  - one-hot assignment via reduce_max + is_ge on DVE reading the PSUM
    scores bank DIRECTLY (no ScalarE eviction hop — DVE is 1x-mode for
    both ops anyway, ~600ns/op measured). rowmax is fp32: a bf16 rowmax
    vs fp32 scores compare produces spurious multi-hots (6x the error).
  - segment sums via fp8 DoubleRow PE matmuls (two token-tiles per mm):
    belongs^T [32,tok] @ X [tok,128] into sums_ps[:, :128] and
    belongs^T @ belongs into sums_ps[:, 128:160] (diag -> counts).
Host: reduce the 8 partial [32,160] sums, then the tiny [4,32,64]
MHSA/BN/FFN head in float64 (0.006% of total FLOPs).

Perf notes (v2; v0 baseline 36.1us, v1 regressed to 42.5us):
  - the run is DMA-streaming-bound: 2x ~3.15MB fp8 per core (xt for
    scores, xn for sums — the two layouts are PE-operand-forced; an
    on-chip transpose costs more PE/DVE than the extra DMA traffic).
  - xt/xn are HOST-PACKED into ONE interleaved dram tensor (per-group
    2048-col xt block | 2048-col xn block) streamed on a single HWDGE
    ring: two concurrent queues measured ~300 GB/s combined vs ~420
    solo (HBM interleaving penalty), and one ring halves the NX issue
    load. mc/out ride the other ring.
  - the PE is warmed with ~3.5us of dummy matmuls during the DMA-arm
    dead time: HAM starts kernels at 1.2 GHz and only unthrottles
    after ~3.4us of sustained busy; in v1 the cold PE (1.9us/group vs
    1.31us arrivals) never warmed until 35us and became the critical
    path. Warm PE is ~1.2us/group and hides under the stream.
  - ~7us of measured exec is an NRT-emitted postamble (255 sem clears
    split across engines) + ~1.5us HWDGE arm + ~0.7us const preamble:
    fixed toolchain tax, not addressable from the kernel.
  - walrus here rejects instructions with >1 sem-wait (_split_waits) and
    the Tile exit barrier is lightened (_TC).
"""

import os
import numpy as np
import ml_dtypes

import concourse.bass as bass
import concourse.mybir as mybir
import concourse.tile as tile
from concourse import bass_utils

B, T, N, C = 4, 12, 4096, 128
L, D, H = 32, 64, 4
HD = D // H
EPS_BN = 1e-5

NCORES = 8
TOK = T * N  # tokens per batch = 49152
TOK_PER_CORE = B * TOK // NCORES  # 24576
TILE_T = 128
NTILE = TOK_PER_CORE // TILE_T  # 192
GT = 16  # token-tiles per scores group (PSUM bank = [128, 512] fp32)
NG = NTILE // GT  # 12
GW = GT * TILE_T  # xt cols per group = 2048
GX = GT * C  # xn cols per group = 2048
GP = GW + GX  # packed cols per group = 4096
W = 160  # sums PSUM width: 128 X-channels + 32 counts (diag of b^T b)
N_WARM = 8  # dummy matmuls to lift HAM to K=8/8 before real work

BF16 = mybir.dt.bfloat16
FP8 = mybir.dt.float8e4
F32 = mybir.dt.float32
_bf = ml_dtypes.bfloat16
_f8 = ml_dtypes.float8_e4m3

_cache = {}


def _split_waits(nc, limit=1):
    """Walrus in this container rejects >1 sem-wait per instruction
    (CoreV3 setupSyncWait): hoist excess waits onto preceding same-engine
    NOPs."""
    n = 0
    for f in nc.m.functions:
        for bb in f.blocks:
            insts = bb.instructions
            i = 0
            while i < len(insts):
                inst = insts[i]
                si = getattr(inst, "sync_info", None)
                if si is not None and si.on_wait is not None and len(si.on_wait) > limit:
                    waits = list(si.on_wait)
                    si.on_wait = waits[:limit]
                    extra = waits[limit:]
                    pos = i
                    while extra:
                        chunk, extra = extra[:limit], extra[limit:]
                        n += 1
                        insts.insert(
                            pos,
                            mybir.InstNoOp(
                                name=f"I-waitsplit-{n}",
                                sync_info=mybir.SyncInfo(on_wait=chunk, on_update=[]),
                                bass_nofuse=True,
                                engine=inst.engine,
                            ),
                        )
                        pos += 1
                        i += 1
                i += 1
    return n


class _TC(tile.TileContext):
    """TileContext with a lighter exit: drop the trailing all-engine
    barrier after the semaphore clears. The clears still run (re-execution
    safe); NRT completion waits for every engine to halt regardless."""

    def _drain_and_barrier(self, tick_clock, wait_clock):
        from concourse.vector_clock import ScopedClock

        drain_inst = self.nc.sync.drain()
        wait_clock.add_sem_waits(
            drain_inst.ins, ScopedClock({None: tick_clock.global_clock})
        )
        self.nc.all_engine_barrier()
        popped = self.nc._tile_sem_poison_stack.pop()
        assert popped is self._sem_poison
        self.nc.clear_and_free_semaphores(list(self.sems.allocated().values()))


def _build_kernel():
    nc = bass.Bass()
    xin = nc.dram_tensor("xin", [TILE_T, NG * GP], FP8, kind="ExternalInput")
    # mc packs M ([:, :L]) and the 16x-tiled c0 row (row 0, cols L:L+GT*L)
    mc = nc.dram_tensor("mc", [C, L + GT * L], FP8, kind="ExternalInput")
    out = nc.dram_tensor("out", [L, W], F32, kind="ExternalOutput")

    with _TC(nc) as tc:
        with (
            tc.tile_pool(name="const", bufs=1) as constp,
            tc.tile_pool(name="xin", bufs=1) as xinp,
            tc.tile_pool(name="work", bufs=5) as workp,
            tc.tile_pool(name="pss", bufs=6, space="PSUM") as pssp,
            tc.tile_pool(name="psum_acc", bufs=1, space="PSUM") as psap,
            tc.tile_pool(name="psum_warm", bufs=1, space="PSUM") as pswp,
        ):
            # mc rides the ACT ring (scores need M + c0 before anything
            # else; 70KB, lands before the first xin slab finishes)
            mc_sb = constp.tile([C, L + GT * L], FP8)
            nc.scalar.dma_start(mc_sb[:], mc[:])
            m_sb = mc_sb[:, :L]
            c0_sb = mc_sb[0:1, L : L + GT * L]
            ones_sb = constp.tile([1, TILE_T], FP8)
            nc.vector.memset(ones_sb[:], 1.0)

            # PE warmup: HAM holds the PE at 1.2 GHz until it sees ~3.4us
            # of sustained activity. Burn the DMA-arm dead time on dummy
            # matmuls so the real stream runs at 2.4 GHz from the start.
            wu_sb = constp.tile([TILE_T, 640], FP8)
            nc.vector.memset(wu_sb[:], 1.0)
            wu_ps = pswp.tile([TILE_T, 512], F32)
            for _ in range(N_WARM):
                nc.tensor.matmul(
                    wu_ps[:],
                    wu_sb[:, :TILE_T],
                    wu_sb[:, TILE_T:],
                    start=True,
                    stop=True,
                    skip_group_check=True,
                )

            # single full-size resident input tile; per-group slab DMAs on
            # ONE ring (solo queue streams ~420 GB/s vs ~300 for two).
            xin_sb = xinp.tile([TILE_T, NG * GP], FP8)
            slabs = [(0, GW), (GW, GP)]  # group 0 split: xt block, xn block
            s = GP
            for span in (1, 2, 2, 2, 2, 1, 1):  # groups 1..11
                slabs.append((s, s + span * GP))
                s += span * GP
            assert s == NG * GP
            for a, b in slabs:
                nc.sync.dma_start(xin_sb[:, a:b], xin[:, a:b])

            sums_ps = psap.tile([L, W], F32)

            def scores_group(g):
                xt_s = xin_sb[:, g * GP : g * GP + GW]
                scores_ps = pssp.tile([TILE_T, GT * L], F32)
                # seed every token row of the group's PSUM with c0
                nc.tensor.matmul(
                    scores_ps[:],
                    ones_sb[:],
                    c0_sb[:],
                    start=True,
                    stop=False,
                    skip_group_check=True,
                )
                for i in range(GT):
                    nc.tensor.matmul(
                        scores_ps[:, i * L : (i + 1) * L],
                        xt_s[:, i * TILE_T : (i + 1) * TILE_T],
                        m_sb[:],
                        start=False,
                        stop=(i == GT - 1),
                        skip_group_check=True,
                    )
                # one-hot straight off PSUM: both DVE ops are 1x-mode
                # regardless (reduce has no 2x uop; is_ge's broadcast
                # operand forces 1x), so an SBUF eviction hop buys nothing
                s3 = scores_ps.rearrange("p (g l) -> p g l", l=L)
                rowmax = workp.tile([TILE_T, GT], F32, tag="rowmax")
                nc.vector.reduce_max(rowmax[:], s3, axis=mybir.AxisListType.X)
                belongs = workp.tile([TILE_T, GT * L], FP8, tag="belongs")
                nc.vector.tensor_tensor(
                    belongs.rearrange("p (g l) -> p g l", l=L),
                    s3,
                    rowmax[:, :, None].to_broadcast((TILE_T, GT, L)),
                    mybir.AluOpType.is_ge,
                )
                return belongs

            def sums_group(g, belongs):
                # fp8 DoubleRow: two token-tiles per matmul (K=256).
                # rhs #1: X tiles -> sums_ps[:, :128]
                # rhs #2: belongs itself -> sums_ps[:, 128:160] (diag=counts)
                b3 = belongs.rearrange("p (k l) -> p k l", l=L)
                x3 = xin_sb[:, g * GP + GW : (g + 1) * GP].rearrange(
                    "p (k w) -> p k w", w=C
                )
                for i in range(GT // 2):
                    first = g == 0 and i == 0
                    last = g == NG - 1 and i == GT // 2 - 1
                    nc.tensor.matmul(
                        sums_ps[:, :C],
                        b3[:, 2 * i : 2 * i + 2, :],
                        x3[:, 2 * i : 2 * i + 2, :],
                        start=first,
                        stop=False,
                        perf_mode=mybir.MatmulPerfMode.DoubleRow,
                        skip_group_check=True,
                    )
                    nc.tensor.matmul(
                        sums_ps[:, C : C + L],
                        b3[:, 2 * i : 2 * i + 2, :],
                        b3[:, 2 * i : 2 * i + 2, :],
                        start=False,
                        stop=last,
                        perf_mode=mybir.MatmulPerfMode.DoubleRow,
                        skip_group_check=True,
                    )

            # software pipeline: sums-matmuls run THREE groups behind the
            # scores-matmuls. The PE queue is strict FIFO for matmuls: a
            # sums MM waiting on the DVE one-hot at the queue head blocks
            # every later matmul, so the skew must exceed the DVE latency
            # (~1.4us) in group-cycles or the whole pipeline serializes
            # (v2 measured 2.7us/group of PE-DVE ping-pong at skew 1).
            pend = []
            for g in range(NG):
                pend.append((g, scores_group(g)))
                if len(pend) > 3:
                    pg, cur = pend.pop(0)
                    sums_group(pg, cur)
            for pg, cur in pend:
                sums_group(pg, cur)

            out_sb = constp.tile([L, W], F32, tag="out_sb")
            nc.scalar.activation(
                out_sb[:], sums_ps[:], mybir.ActivationFunctionType.Copy
            )
            nc.scalar.dma_start(out[:], out_sb[:])

    _split_waits(nc)
    return nc


def _prep_inputs(STFeature, centroids, Wq_c, bq_c, Wk_n, bk_n):
    X = np.ascontiguousarray(STFeature.reshape(B, TOK, C), dtype=np.float32)
    Qc = centroids.astype(np.float64) @ Wq_c.astype(np.float64) + bq_c.astype(
        np.float64
    )  # [B,L,C]
    M = np.einsum("cj,blj->bcl", Wk_n.astype(np.float64), Qc)  # [B,C,L]
    c0 = np.einsum("j,blj->bl", bk_n.astype(np.float64), Qc)  # [B,L]

    in_maps = []
    for core in range(NCORES):
        b, h = core // 2, core % 2
        rows = X[b][h * TOK_PER_CORE : (h + 1) * TOK_PER_CORE]  # [24576, 128]
        xt = np.ascontiguousarray(rows.T).astype(_f8)  # [128, 24576]
        xn = (
            rows.reshape(NTILE, TILE_T, C).transpose(1, 0, 2).astype(_f8)
        )  # [128, 192, 128]
        # interleave per group: [xt block 2048 | xn block 2048] x 12
        xin = np.concatenate(
            [xt.reshape(TILE_T, NG, GW), xn.reshape(TILE_T, NG, GX)], axis=2
        ).reshape(TILE_T, NG * GP)
        mc = np.zeros((C, L + GT * L), dtype=_f8)
        mc[:, :L] = M[b].astype(np.float32).astype(_f8)
        mc[0, L:] = np.tile(c0[b].astype(np.float32).astype(_f8), GT)
        in_maps.append({"xin": np.ascontiguousarray(xin), "mc": mc})
    return in_maps


def _small_path(Xsum, counts, centroids, Wv_n, bv_n, Wal, bal, Wq, bq, Wk, bk, Wv, bv,
                Wo, bo, bn_gamma, bn_beta, alpha, beta, W1, b1, W2, b2):
    f = lambda a: np.asarray(a, np.float64)
    V = Xsum @ f(Wv_n) + counts[:, :, None] * f(bv_n)
    cluster = V / (counts**2 + 1.0)[:, :, None]
    cen = f(centroids) + cluster @ f(Wal) + f(bal)
    q = (cen @ f(Wq) + f(bq)).reshape(B, L, H, HD).transpose(0, 2, 1, 3)
    k = (cen @ f(Wk) + f(bk)).reshape(B, L, H, HD).transpose(0, 2, 1, 3)
    v = (cen @ f(Wv) + f(bv)).reshape(B, L, H, HD).transpose(0, 2, 1, 3)
    s = np.einsum("bhld,bhmd->bhlm", q, k) / np.sqrt(np.float64(HD))
    s = s - s.max(axis=-1, keepdims=True)
    e = np.exp(s)
    attn = e / e.sum(axis=-1, keepdims=True)
    a = np.einsum("bhlm,bhmd->bhld", attn, v).transpose(0, 2, 1, 3).reshape(B, L, D)
    a = a @ f(Wo) + f(bo)
    z = cen + a
    mu = z.mean(axis=(0, 1))
    var = z.var(axis=(0, 1))
    z = (z - mu) / np.sqrt(var + EPS_BN) * f(bn_gamma) + f(bn_beta)
    z = f(alpha) * z + f(beta)
    return np.maximum(z @ f(W1) + f(b1), 0.0) @ f(W2) + f(b2)


def kernel(**inputs):
    inputs = {k: np.asarray(v) for k, v in inputs.items()}
    in_maps = _prep_inputs(
        inputs["STFeature"].astype(np.float32),
        inputs["centroids"],
        inputs["Wq_c"],
        inputs["bq_c"],
        inputs["Wk_n"],
        inputs["bk_n"],
    )

    if "nc" not in _cache:
        _cache["nc"] = _build_kernel()
    nc = _cache["nc"]

    run_kwargs = {}
    if os.environ.get("CLUSF_TRACE"):
        run_kwargs = {"trace": True, "tmpdir": os.environ.get("CLUSF_TRACE_DIR")}
    res = bass_utils.run_bass_kernel_spmd(
        nc, in_maps, core_ids=list(range(NCORES)), **run_kwargs
    )
    _cache["last_result"] = res

    sums8 = np.stack([res.results[i]["out"] for i in range(NCORES)])  # [8,32,W]
    S = (sums8[0::2] + sums8[1::2]).astype(np.float64)  # [B,32,W]
    Xsum = S[:, :, :C]
    counts = np.einsum("bll->bl", S[:, :, C : C + L])  # diag of belongs^T belongs

    out = _small_path(
        Xsum, counts,
        inputs["centroids"], inputs["Wv_n"], inputs["bv_n"], inputs["Wal"],
        inputs["bal"], inputs["Wq"], inputs["bq"], inputs["Wk"], inputs["bk"],
        inputs["Wv"], inputs["bv"], inputs["Wo"], inputs["bo"],
        inputs["bn_gamma"], inputs["bn_beta"], inputs["alpha"], inputs["beta"],
        inputs["W1"], inputs["b1"], inputs["W2"], inputs["b2"],
    )
    return out.astype(np.float32)
